# revision 38
# baseline (speedup 1.0000x reference)
"""Trainium2 Bass kernel: segmented attention with compressive memory
(Infini-attention style). 8-core SPMD: 32 (b,h) pairs sharded 4/core.

Host prepares layout-optimized bf16 inputs (rope applied, elu-sigma
applied, transposed copies); device does all matmuls, softmax,
the d x d memory recurrence, gating and output assembly.
"""
import sys
import numpy as np

sys.path.insert(0, "/opt/trn_rl_repo")

import ml_dtypes  # noqa: E402

BF16 = ml_dtypes.bfloat16

B, H, S, D = 4, 8, 8192, 64
SEG = 1024
NSEG = S // SEG
NPAIR_CORE = 4          # (b,h) pairs per core
NCORES = 8
EPS = 1e-6
ROPE_THETA = 10000.0

_GRAPH_CACHE = {}


def _rope_tables():
    inv_freq = 1.0 / (ROPE_THETA ** (np.arange(0, D, 2, dtype=np.float32) / D))
    t = np.arange(SEG, dtype=np.float32)
    freqs = np.einsum("i,j->ij", t, inv_freq)
    emb = np.concatenate([freqs, freqs], axis=-1)   # [SEG, D]
    return np.cos(emb).astype(np.float32), np.sin(emb).astype(np.float32)


def _apply_rope_np(x, cos, sin):
    # x: [P, NSEG, SEG, D]
    x1, x2 = x[..., : D // 2], x[..., D // 2:]
    rot = np.concatenate([-x2, x1], axis=-1)
    return x * cos + rot * sin


def _build_graph():
    if "nc" in _GRAPH_CACHE:
        return _GRAPH_CACHE["nc"], _GRAPH_CACHE["names"]

    import concourse.bass as bass
    import concourse.tile as tile
    from concourse import bacc, mybir

    f32 = mybir.dt.float32
    bf16 = mybir.dt.bfloat16
    MULT = mybir.AluOpType.mult
    DIV = mybir.AluOpType.divide
    ADD = mybir.AluOpType.add

    nc = bacc.Bacc(
        "TRN2",
        target_bir_lowering=False,
        debug=False,
        enable_asserts=False,
        num_devices=NCORES,
    )

    # per-core DRAM inputs (host-prepped layouts)
    # qkq: stacked [pair, {qrT,krT,sqT}, D, S]
    qkq = nc.dram_tensor("qkq", (NPAIR_CORE, 3, D, S), bf16, kind="ExternalInput").ap()
    # pre-tiled [pair, seg, 128, 8*64]
    skt = nc.dram_tensor("skt", (NPAIR_CORE, NSEG, 128, 512), bf16, kind="ExternalInput").ap()
    vt = nc.dram_tensor("vt", (NPAIR_CORE, NSEG, 128, 512), bf16, kind="ExternalInput").ap()
    ident = nc.dram_tensor("ident", (128, 128), bf16, kind="ExternalInput").ap()
    mask = nc.dram_tensor("mask", (128, 128), bf16, kind="ExternalInput").ap()
    gsc = nc.dram_tensor("gsc", (128, 2 * NPAIR_CORE), f32, kind="ExternalInput").ap()
    out = nc.dram_tensor("out", (NPAIR_CORE, S, D), f32, kind="ExternalOutput").ap()

    with tile.TileContext(nc) as tc:
        with (
            tc.tile_pool(name="consts", bufs=1) as consts,
            tc.tile_pool(name="qk_in", bufs=3) as qk_in,
            tc.tile_pool(name="kv_in", bufs=3) as kv_in,
            tc.tile_pool(name="pt", bufs=3) as ptp,
            tc.tile_pool(name="cp", bufs=2) as cpp,
            tc.tile_pool(name="msnap", bufs=2) as msnapp,
            tc.tile_pool(name="outsb", bufs=3) as outsb,
            tc.tile_pool(name="ps_m", bufs=1, space="PSUM") as ps_m,
            tc.tile_pool(name="ps_mem", bufs=1, space="PSUM") as ps_mem,
            tc.tile_pool(name="ps_att", bufs=1, space="PSUM") as ps_att,
            tc.tile_pool(name="ps_st", bufs=3, space="PSUM") as ps_st,
        ):
            mkt = consts.tile([128, 128], bf16)
            nc.sync.dma_start(mkt[:], mask[:])
            gst = consts.tile([128, 2 * NPAIR_CORE], f32)
            nc.sync.dma_start(gst[:], gsc[:])
            magic = consts.tile([128, 16], mybir.dt.int32)
            nc.gpsimd.memset(magic[:], 0x7EF311C3)

            for p in range(NPAIR_CORE):
                m_aug = ps_m.tile([D, D + 1], f32)       # [M | norm] accumulator
                m_snap = msnapp.tile([D, D + 1], bf16)
                nc.gpsimd.memset(m_snap[:], 0.0)
                g_col = gst[:, p : p + 1]
                omg_col = gst[:, NPAIR_CORE + p : NPAIR_CORE + p + 1]

                for s in range(NSEG):
                    # q/k duplicated into both partition halves so K=64 score
                    # matmuls row-pack into PE row-groups 0-1 / 2-3
                    qkq_t = qk_in.tile([128, 3, SEG], bf16, tag="qkq")
                    src = qkq[p, :, :, s * SEG : (s + 1) * SEG].rearrange(
                        "c d n -> d c n"
                    )
                    nc.sync.dma_start(qkq_t[0:D, :, :], src)
                    nc.sync.dma_start(qkq_t[D : 2 * D, :, :], src)
                    q_h = [qkq_t[0:D, 0, :], qkq_t[D : 2 * D, 0, :]]
                    k_h = [qkq_t[0:D, 1, :], qkq_t[D : 2 * D, 1, :]]
                    sq_t = qkq_t[0:D, 2, :]
                    sk_t = kv_in.tile([128, 8, 64], bf16, tag="sk")
                    nc.sync.dma_start(sk_t[:], skt[p, s].rearrange("p (t d) -> p t d", t=8))
                    v_aug = kv_in.tile([128, 8, 65], bf16, tag="v")
                    nc.sync.dma_start(
                        v_aug[:, :, 0:64], vt[p, s].rearrange("p (t d) -> p t d", t=8)
                    )
                    nc.vector.memset(v_aug[:, :, 64:65], 1.0)

                    # ---- memory update: M_aug += sigma_k^T @ [v | 1]
                    for t in range(8):
                        nc.tensor.matmul(
                            m_aug[:],
                            sk_t[:, t, :],
                            v_aug[:, t, :],
                            start=(s == 0 and t == 0),
                            stop=(s == NSEG - 1 and t == 7),
                            skip_group_check=True,
                        )

                    # ---- S^T = Kr @ Qr^T (causal chunks), exp, mask diag
                    pt = ptp.tile([128, 8, SEG], bf16)
                    for t in range(8):
                        chunks = []
                        if t < 4:
                            chunks.append((t * 128, 512))
                        chunks.append((max(t * 128, 512), SEG))
                        h = t % 2
                        for (c0, c1) in chunks:
                            st = ps_st.tile([128, 512], f32, tag="st")
                            nc.tensor.matmul(
                                st[:, 0 : c1 - c0],
                                k_h[h][:, t * 128 : (t + 1) * 128],
                                q_h[h][:, c0:c1],
                                start=True,
                                stop=True,
                                skip_group_check=True,
                                tile_position=(h * D, 0),
                            )
                            nc.scalar.activation(
                                pt[:, t, c0:c1],
                                st[:, 0 : c1 - c0],
                                mybir.ActivationFunctionType.Exp,
                            )
                        nc.vector.tensor_tensor(
                            pt[:, t, t * 128 : (t + 1) * 128],
                            pt[:, t, t * 128 : (t + 1) * 128],
                            mkt[:],
                            op=MULT,
                        )

                    # ---- per 128-q chunk: PV and memory retrieval directly in
                    # [q, 65] layout (lhsT = P^T chunk / sigma_q^T chunk)
                    att_big = ps_att.tile([128, 8, 128], f32)
                    mem_big = None
                    if s > 0:
                        mem_big = ps_mem.tile([128, 8, 128], f32)
                    for j in range(8):
                        for t in range(j + 1):
                            nc.tensor.matmul(
                                att_big[:, j, 0 : D + 1],
                                pt[:, t, j * 128 : (j + 1) * 128],
                                v_aug[:, t, :],
                                start=(t == 0),
                                stop=(t == j),
                                skip_group_check=True,
                            )
                        if s > 0:
                            nc.tensor.matmul(
                                mem_big[:, j, 0 : D + 1],
                                sq_t[:, j * 128 : (j + 1) * 128],
                                m_snap[:],
                                start=True, stop=True, skip_group_check=True,
                            )

                    dens = outsb.tile([128, 16], f32, tag="dens")
                    nc.vector.tensor_copy(dens[:, 0:8], att_big[:, :, D])
                    if s > 0:
                        nc.vector.tensor_scalar(
                            dens[:, 8:16], mem_big[:, :, D], EPS, None, op0=ADD
                        )
                    else:
                        nc.vector.memset(dens[:, 8:16], 1.0)
                    # Newton reciprocal: seed from exponent bits, 2 iters
                    recs = outsb.tile([128, 16], f32, tag="recs")
                    nc.vector.tensor_tensor(
                        recs[:].bitcast(mybir.dt.int32), magic[:],
                        dens[:].bitcast(mybir.dt.int32),
                        op=mybir.AluOpType.subtract,
                    )
                    nwt = outsb.tile([128, 16], f32, tag="nwt")
                    for _ in range(2):
                        nc.vector.tensor_tensor(nwt[:], dens[:], recs[:], op=MULT)
                        nc.vector.tensor_scalar(
                            nwt[:], nwt[:], -1.0, 2.0, op0=MULT, op1=ADD
                        )
                        nc.vector.tensor_tensor(recs[:], recs[:], nwt[:], op=MULT)
                    # fold gates into the per-chunk reciprocals
                    recs_a = outsb.tile([128, 8], f32, tag="ra")
                    nc.vector.tensor_scalar(recs_a[:], recs[:, 0:8], omg_col, None, op0=MULT)
                    recs_m = outsb.tile([128, 8], f32, tag="rm")
                    nc.vector.tensor_scalar(recs_m[:], recs[:, 8:16], g_col, None, op0=MULT)

                    o_sb = outsb.tile([128, 8, D], f32, tag="o")
                    if s > 0:
                        t1 = outsb.tile([128, 8, D], f32, tag="t1")
                        nc.vector.tensor_tensor(
                            t1[:], att_big[:, :, 0:D],
                            recs_a[:].unsqueeze(2).broadcast_to([128, 8, D]),
                            op=MULT,
                        )
                        t2 = outsb.tile([128, 8, D], f32, tag="t2")
                        nc.vector.tensor_tensor(
                            t2[:], mem_big[:, :, 0:D],
                            recs_m[:].unsqueeze(2).broadcast_to([128, 8, D]),
                            op=MULT,
                        )
                        nc.vector.tensor_tensor(o_sb[:], t1[:], t2[:], op=ADD)
                    else:
                        # segment 0: memory is empty, output is (1-g)*attn
                        nc.vector.tensor_tensor(
                            o_sb[:], att_big[:, :, 0:D],
                            recs_a[:].unsqueeze(2).broadcast_to([128, 8, D]),
                            op=MULT,
                        )
                    nc.sync.dma_start(
                        out[p, s * SEG : (s + 1) * SEG, :].rearrange(
                            "(jj pp) d -> pp jj d", pp=128
                        ),
                        o_sb[:],
                    )

                    # ---- snapshot memory state for next segment
                    if s < NSEG - 1:
                        m_snap = msnapp.tile([D, D + 1], bf16)
                        nc.vector.tensor_copy(m_snap[:], m_aug[:])

    nc.compile()
    names = dict(qrT="qrT", krT="krT", sqT="sqT", skt="skt", vt="vt",
                 ident="ident", mask="mask", gsc="gsc", out="out")
    _GRAPH_CACHE["nc"] = nc
    _GRAPH_CACHE["names"] = names
    return nc, names


def _host_prep(q, k, v, gate):
    """Produce per-core input maps."""
    cos, sin = _rope_tables()
    P = B * H
    qp = q.reshape(P, NSEG, SEG, D).astype(np.float32)
    kp = k.reshape(P, NSEG, SEG, D).astype(np.float32)
    vp = v.reshape(P, S, D).astype(np.float32)

    qr = _apply_rope_np(qp, cos, sin) * np.float32(1.0 / np.sqrt(D))
    kr = _apply_rope_np(kp, cos, sin)
    sq = np.where(qp > 0, qp + 1.0, np.exp(np.minimum(qp, 0.0))).astype(np.float32)
    sk = np.where(kp > 0, kp + 1.0, np.exp(np.minimum(kp, 0.0))).astype(np.float32)
    # stacked + transposed [P, 3, D, S]
    qkq = np.ascontiguousarray(
        np.stack(
            [qr.reshape(P, S, D), kr.reshape(P, S, D), sq.reshape(P, S, D)],
            axis=1,
        ).transpose(0, 1, 3, 2)
    ).astype(BF16)
    # pre-tiled [pair, seg, 128, 8*64]
    skt = np.ascontiguousarray(
        sk.reshape(P, NSEG, 8, 128, D).transpose(0, 1, 3, 2, 4)
        .reshape(P, NSEG, 128, 512)).astype(BF16)
    vt = np.ascontiguousarray(
        vp.reshape(P, NSEG, 8, 128, D).transpose(0, 1, 3, 2, 4)
        .reshape(P, NSEG, 128, 512)).astype(BF16)

    ident = np.eye(128, dtype=np.float32).astype(BF16)
    mask = np.triu(np.ones((128, 128), dtype=np.float32)).astype(BF16)

    g = 1.0 / (1.0 + np.exp(-gate.reshape(H).astype(np.float64)))
    g = g.astype(np.float32)

    in_maps = []
    for c in range(NCORES):
        pairs = range(c * NPAIR_CORE, (c + 1) * NPAIR_CORE)
        gsc = np.zeros((128, 2 * NPAIR_CORE), dtype=np.float32)
        for i, pr in enumerate(pairs):
            gsc[:, i] = g[pr % H]
            gsc[:, NPAIR_CORE + i] = 1.0 - g[pr % H]
        sl = slice(c * NPAIR_CORE, (c + 1) * NPAIR_CORE)
        in_maps.append({
            "qkq": qkq[sl], "skt": skt[sl], "vt": vt[sl],
            "ident": ident, "mask": mask, "gsc": gsc,
        })
    return in_maps


def kernel(q, k, v, gate, _trace=False):
    from concourse import bass_utils

    nc, _ = _build_graph()
    in_maps = _host_prep(q, k, v, gate)
    res = bass_utils.run_bass_kernel_spmd(
        nc, in_maps, core_ids=list(range(NCORES)), trace=_trace
    )
    outs = [res.results[c]["out"] for c in range(NCORES)]
    full = np.concatenate(outs, axis=0).reshape(B, H, S, D).astype(np.float32)
    if _trace:
        kernel.last_exec_time_ns = res.exec_time_ns
        kernel.last_results = res
    return full


# revision 41
# speedup vs baseline: 1.2171x; 1.2171x over previous
"""Trainium2 Bass kernel: segmented attention with compressive memory
(Infini-attention style). 8-core SPMD: 32 (b,h) pairs sharded 4/core.

Host prepares layout-optimized bf16 inputs (rope applied, elu-sigma
applied, transposed copies); device does all matmuls, softmax,
the d x d memory recurrence, gating and output assembly.
"""
import sys
import numpy as np

sys.path.insert(0, "/opt/trn_rl_repo")

import ml_dtypes  # noqa: E402

BF16 = ml_dtypes.bfloat16

B, H, S, D = 4, 8, 8192, 64
SEG = 1024
NSEG = S // SEG
NPAIR_CORE = 4          # (b,h) pairs per core
NCORES = 8
EPS = 1e-6
ROPE_THETA = 10000.0

_GRAPH_CACHE = {}


def _rope_tables():
    inv_freq = 1.0 / (ROPE_THETA ** (np.arange(0, D, 2, dtype=np.float32) / D))
    t = np.arange(SEG, dtype=np.float32)
    freqs = np.einsum("i,j->ij", t, inv_freq)
    emb = np.concatenate([freqs, freqs], axis=-1)   # [SEG, D]
    return np.cos(emb).astype(np.float32), np.sin(emb).astype(np.float32)


def _apply_rope_np(x, cos, sin):
    # x: [P, NSEG, SEG, D]
    x1, x2 = x[..., : D // 2], x[..., D // 2:]
    rot = np.concatenate([-x2, x1], axis=-1)
    return x * cos + rot * sin


def _build_graph():
    if "nc" in _GRAPH_CACHE:
        return _GRAPH_CACHE["nc"], _GRAPH_CACHE["names"]

    import concourse.bass as bass
    import concourse.tile as tile
    from concourse import bacc, mybir

    f32 = mybir.dt.float32
    bf16 = mybir.dt.bfloat16
    MULT = mybir.AluOpType.mult
    DIV = mybir.AluOpType.divide
    ADD = mybir.AluOpType.add

    nc = bacc.Bacc(
        "TRN2",
        target_bir_lowering=False,
        debug=False,
        enable_asserts=False,
        num_devices=NCORES,
    )

    # per-core DRAM inputs (host-prepped layouts)
    # qkq: stacked [pair, {qrT,krT,sqT}, D, S]
    qkq = nc.dram_tensor("qkq", (NPAIR_CORE, 3, D, S), bf16, kind="ExternalInput").ap()
    # pre-tiled [pair, seg, 128, 8*64]
    skt = nc.dram_tensor("skt", (NPAIR_CORE, NSEG, 128, 512), bf16, kind="ExternalInput").ap()
    vt = nc.dram_tensor("vt", (NPAIR_CORE, NSEG, 128, 512), bf16, kind="ExternalInput").ap()
    ident = nc.dram_tensor("ident", (128, 128), bf16, kind="ExternalInput").ap()
    mask = nc.dram_tensor("mask", (128, 128), bf16, kind="ExternalInput").ap()
    gsc = nc.dram_tensor("gsc", (128, 2 * NPAIR_CORE), f32, kind="ExternalInput").ap()
    out = nc.dram_tensor("out", (NPAIR_CORE, S, D), f32, kind="ExternalOutput").ap()

    with tile.TileContext(nc) as tc:
        with (
            tc.tile_pool(name="consts", bufs=1) as consts,
            tc.tile_pool(name="qk_in", bufs=3) as qk_in,
            tc.tile_pool(name="kv_in", bufs=3) as kv_in,
            tc.tile_pool(name="pt", bufs=3) as ptp,
            tc.tile_pool(name="cp", bufs=2) as cpp,
            tc.tile_pool(name="msnap", bufs=2) as msnapp,
            tc.tile_pool(name="outsb", bufs=3) as outsb,
            tc.tile_pool(name="ps_m", bufs=1, space="PSUM") as ps_m,
            tc.tile_pool(name="ps_mem", bufs=1, space="PSUM") as ps_mem,
            tc.tile_pool(name="ps_att", bufs=1, space="PSUM") as ps_att,
            tc.tile_pool(name="ps_st", bufs=3, space="PSUM") as ps_st,
        ):
            mkt = consts.tile([128, 128], bf16)
            nc.sync.dma_start(mkt[:], mask[:])
            gst = consts.tile([128, 2 * NPAIR_CORE], f32)
            nc.sync.dma_start(gst[:], gsc[:])
            magic = consts.tile([128, 16], mybir.dt.int32)
            nc.gpsimd.memset(magic[:], 0x7EF311C3)

            for p in range(NPAIR_CORE):
                m_aug = ps_m.tile([D, D + 1], f32)       # [M | norm] accumulator
                m_snap = msnapp.tile([D, D + 1], bf16)
                nc.gpsimd.memset(m_snap[:], 0.0)
                g_col = gst[:, p : p + 1]
                omg_col = gst[:, NPAIR_CORE + p : NPAIR_CORE + p + 1]

                for s in range(NSEG):
                    qkq_t = qk_in.tile([D, 3, SEG], bf16, tag="qkq")
                    nc.sync.dma_start(
                        qkq_t[:],
                        qkq[p, :, :, s * SEG : (s + 1) * SEG].rearrange(
                            "c d n -> d c n"
                        ),
                    )
                    q_t = qkq_t[:, 0, :]
                    k_t = qkq_t[:, 1, :]
                    sq_t = qkq_t[:, 2, :]
                    sk_t = kv_in.tile([128, 8, 64], bf16, tag="sk")
                    nc.sync.dma_start(sk_t[:], skt[p, s].rearrange("p (t d) -> p t d", t=8))
                    v_aug = kv_in.tile([128, 8, 65], bf16, tag="v")
                    nc.sync.dma_start(
                        v_aug[:, :, 0:64], vt[p, s].rearrange("p (t d) -> p t d", t=8)
                    )
                    nc.vector.memset(v_aug[:, :, 64:65], 1.0)

                    # ---- S^T = Kr @ Qr^T (causal chunks), exp, mask diag
                    pt = ptp.tile([128, 8, SEG], bf16)
                    for t in range(8):
                        chunks = []
                        if t < 4:
                            chunks.append((t * 128, 512))
                        chunks.append((max(t * 128, 512), SEG))
                        for (c0, c1) in chunks:
                            st = ps_st.tile([128, 512], f32, tag="st")
                            nc.tensor.matmul(
                                st[:, 0 : c1 - c0],
                                k_t[:, t * 128 : (t + 1) * 128],
                                q_t[:, c0:c1],
                                start=True,
                                stop=True,
                                skip_group_check=True,
                            )
                            nc.scalar.activation(
                                pt[:, t, c0:c1],
                                st[:, 0 : c1 - c0],
                                mybir.ActivationFunctionType.Exp,
                            )
                        nc.vector.tensor_tensor(
                            pt[:, t, t * 128 : (t + 1) * 128],
                            pt[:, t, t * 128 : (t + 1) * 128],
                            mkt[:],
                            op=MULT,
                        )

                    # ---- memory update: M_aug += sigma_k^T @ [v | 1]
                    # (emitted after S^T so it fills PE gaps during exp waits)
                    for t in range(8):
                        nc.tensor.matmul(
                            m_aug[:],
                            sk_t[:, t, :],
                            v_aug[:, t, :],
                            start=(s == 0 and t == 0),
                            stop=(s == NSEG - 1 and t == 7),
                            skip_group_check=True,
                        )

                    # ---- per 128-q chunk: PV and memory retrieval directly in
                    # [q, 65] layout (lhsT = P^T chunk / sigma_q^T chunk)
                    att_big = ps_att.tile([128, 8, 128], f32)
                    mem_big = None
                    if s > 0:
                        mem_big = ps_mem.tile([128, 8, 128], f32)
                    for j in range(8):
                        for t in range(j + 1):
                            nc.tensor.matmul(
                                att_big[:, j, 0 : D + 1],
                                pt[:, t, j * 128 : (j + 1) * 128],
                                v_aug[:, t, :],
                                start=(t == 0),
                                stop=(t == j),
                                skip_group_check=True,
                            )
                        if s > 0:
                            nc.tensor.matmul(
                                mem_big[:, j, 0 : D + 1],
                                sq_t[:, j * 128 : (j + 1) * 128],
                                m_snap[:],
                                start=True, stop=True, skip_group_check=True,
                            )

                    dens = outsb.tile([128, 16], f32, tag="dens")
                    nc.vector.tensor_copy(dens[:, 0:8], att_big[:, :, D])
                    if s > 0:
                        nc.vector.tensor_scalar(
                            dens[:, 8:16], mem_big[:, :, D], EPS, None, op0=ADD
                        )
                    else:
                        nc.vector.memset(dens[:, 8:16], 1.0)
                    # Newton reciprocal: seed from exponent bits, 2 iters
                    recs = outsb.tile([128, 16], f32, tag="recs")
                    nc.vector.tensor_tensor(
                        recs[:].bitcast(mybir.dt.int32), magic[:],
                        dens[:].bitcast(mybir.dt.int32),
                        op=mybir.AluOpType.subtract,
                    )
                    nwt = outsb.tile([128, 16], f32, tag="nwt")
                    for _ in range(2):
                        nc.vector.tensor_tensor(nwt[:], dens[:], recs[:], op=MULT)
                        nc.vector.tensor_scalar(
                            nwt[:], nwt[:], -1.0, 2.0, op0=MULT, op1=ADD
                        )
                        nc.vector.tensor_tensor(recs[:], recs[:], nwt[:], op=MULT)
                    # fold gates into the per-chunk reciprocals
                    recs_a = outsb.tile([128, 8], f32, tag="ra")
                    nc.vector.tensor_scalar(recs_a[:], recs[:, 0:8], omg_col, None, op0=MULT)
                    recs_m = outsb.tile([128, 8], f32, tag="rm")
                    nc.vector.tensor_scalar(recs_m[:], recs[:, 8:16], g_col, None, op0=MULT)

                    o_sb = outsb.tile([128, 8, D], f32, tag="o")
                    if s > 0:
                        t1 = outsb.tile([128, 8, D], f32, tag="t1")
                        nc.vector.tensor_tensor(
                            t1[:], att_big[:, :, 0:D],
                            recs_a[:].unsqueeze(2).broadcast_to([128, 8, D]),
                            op=MULT,
                        )
                        t2 = outsb.tile([128, 8, D], f32, tag="t2")
                        nc.vector.tensor_tensor(
                            t2[:], mem_big[:, :, 0:D],
                            recs_m[:].unsqueeze(2).broadcast_to([128, 8, D]),
                            op=MULT,
                        )
                        nc.vector.tensor_tensor(o_sb[:], t1[:], t2[:], op=ADD)
                    else:
                        # segment 0: memory is empty, output is (1-g)*attn
                        nc.vector.tensor_tensor(
                            o_sb[:], att_big[:, :, 0:D],
                            recs_a[:].unsqueeze(2).broadcast_to([128, 8, D]),
                            op=MULT,
                        )
                    nc.sync.dma_start(
                        out[p, s * SEG : (s + 1) * SEG, :].rearrange(
                            "(jj pp) d -> pp jj d", pp=128
                        ),
                        o_sb[:],
                    )

                    # ---- snapshot memory state for next segment
                    if s < NSEG - 1:
                        m_snap = msnapp.tile([D, D + 1], bf16)
                        nc.vector.tensor_copy(m_snap[:], m_aug[:])

    nc.compile()
    names = dict(qrT="qrT", krT="krT", sqT="sqT", skt="skt", vt="vt",
                 ident="ident", mask="mask", gsc="gsc", out="out")
    _GRAPH_CACHE["nc"] = nc
    _GRAPH_CACHE["names"] = names
    return nc, names


def _host_prep(q, k, v, gate):
    """Produce per-core input maps."""
    cos, sin = _rope_tables()
    P = B * H
    qp = q.reshape(P, NSEG, SEG, D).astype(np.float32)
    kp = k.reshape(P, NSEG, SEG, D).astype(np.float32)
    vp = v.reshape(P, S, D).astype(np.float32)

    qr = _apply_rope_np(qp, cos, sin) * np.float32(1.0 / np.sqrt(D))
    kr = _apply_rope_np(kp, cos, sin)
    sq = np.where(qp > 0, qp + 1.0, np.exp(np.minimum(qp, 0.0))).astype(np.float32)
    sk = np.where(kp > 0, kp + 1.0, np.exp(np.minimum(kp, 0.0))).astype(np.float32)
    # stacked + transposed [P, 3, D, S]
    qkq = np.ascontiguousarray(
        np.stack(
            [qr.reshape(P, S, D), kr.reshape(P, S, D), sq.reshape(P, S, D)],
            axis=1,
        ).transpose(0, 1, 3, 2)
    ).astype(BF16)
    # pre-tiled [pair, seg, 128, 8*64]
    skt = np.ascontiguousarray(
        sk.reshape(P, NSEG, 8, 128, D).transpose(0, 1, 3, 2, 4)
        .reshape(P, NSEG, 128, 512)).astype(BF16)
    vt = np.ascontiguousarray(
        vp.reshape(P, NSEG, 8, 128, D).transpose(0, 1, 3, 2, 4)
        .reshape(P, NSEG, 128, 512)).astype(BF16)

    ident = np.eye(128, dtype=np.float32).astype(BF16)
    mask = np.triu(np.ones((128, 128), dtype=np.float32)).astype(BF16)

    g = 1.0 / (1.0 + np.exp(-gate.reshape(H).astype(np.float64)))
    g = g.astype(np.float32)

    in_maps = []
    for c in range(NCORES):
        pairs = range(c * NPAIR_CORE, (c + 1) * NPAIR_CORE)
        gsc = np.zeros((128, 2 * NPAIR_CORE), dtype=np.float32)
        for i, pr in enumerate(pairs):
            gsc[:, i] = g[pr % H]
            gsc[:, NPAIR_CORE + i] = 1.0 - g[pr % H]
        sl = slice(c * NPAIR_CORE, (c + 1) * NPAIR_CORE)
        in_maps.append({
            "qkq": qkq[sl], "skt": skt[sl], "vt": vt[sl],
            "ident": ident, "mask": mask, "gsc": gsc,
        })
    return in_maps


def kernel(q, k, v, gate, _trace=False):
    from concourse import bass_utils

    nc, _ = _build_graph()
    in_maps = _host_prep(q, k, v, gate)
    res = bass_utils.run_bass_kernel_spmd(
        nc, in_maps, core_ids=list(range(NCORES)), trace=_trace
    )
    outs = [res.results[c]["out"] for c in range(NCORES)]
    full = np.concatenate(outs, axis=0).reshape(B, H, S, D).astype(np.float32)
    if _trace:
        kernel.last_exec_time_ns = res.exec_time_ns
        kernel.last_results = res
    return full


# revision 43
# speedup vs baseline: 1.2177x; 1.0005x over previous
"""Trainium2 Bass kernel: segmented attention with compressive memory
(Infini-attention style). 8-core SPMD: 32 (b,h) pairs sharded 4/core.

Host prepares layout-optimized bf16 inputs (rope applied, elu-sigma
applied, transposed copies); device does all matmuls, softmax,
the d x d memory recurrence, gating and output assembly.
"""
import sys
import numpy as np

sys.path.insert(0, "/opt/trn_rl_repo")

import ml_dtypes  # noqa: E402

BF16 = ml_dtypes.bfloat16

B, H, S, D = 4, 8, 8192, 64
SEG = 1024
NSEG = S // SEG
NPAIR_CORE = 4          # (b,h) pairs per core
NCORES = 8
EPS = 1e-6
ROPE_THETA = 10000.0

_GRAPH_CACHE = {}


def _rope_tables():
    inv_freq = 1.0 / (ROPE_THETA ** (np.arange(0, D, 2, dtype=np.float32) / D))
    t = np.arange(SEG, dtype=np.float32)
    freqs = np.einsum("i,j->ij", t, inv_freq)
    emb = np.concatenate([freqs, freqs], axis=-1)   # [SEG, D]
    return np.cos(emb).astype(np.float32), np.sin(emb).astype(np.float32)


def _apply_rope_np(x, cos, sin):
    # x: [P, NSEG, SEG, D]
    x1, x2 = x[..., : D // 2], x[..., D // 2:]
    rot = np.concatenate([-x2, x1], axis=-1)
    return x * cos + rot * sin


def _build_graph():
    if "nc" in _GRAPH_CACHE:
        return _GRAPH_CACHE["nc"], _GRAPH_CACHE["names"]

    import concourse.bass as bass
    import concourse.tile as tile
    from concourse import bacc, mybir

    f32 = mybir.dt.float32
    bf16 = mybir.dt.bfloat16
    MULT = mybir.AluOpType.mult
    DIV = mybir.AluOpType.divide
    ADD = mybir.AluOpType.add

    nc = bacc.Bacc(
        "TRN2",
        target_bir_lowering=False,
        debug=False,
        enable_asserts=False,
        num_devices=NCORES,
    )

    # per-core DRAM inputs (host-prepped layouts)
    # qkq: stacked [pair, {qrT,krT,sqT}, D, S]
    qkq = nc.dram_tensor("qkq", (NPAIR_CORE, 3, D, S), bf16, kind="ExternalInput").ap()
    # pre-tiled [pair, seg, 128, 8*64]
    skt = nc.dram_tensor("skt", (NPAIR_CORE, NSEG, 128, 512), bf16, kind="ExternalInput").ap()
    vt = nc.dram_tensor("vt", (NPAIR_CORE, NSEG, 128, 512), bf16, kind="ExternalInput").ap()
    ident = nc.dram_tensor("ident", (128, 128), bf16, kind="ExternalInput").ap()
    mask = nc.dram_tensor("mask", (128, 128), bf16, kind="ExternalInput").ap()
    gsc = nc.dram_tensor("gsc", (128, 2 * NPAIR_CORE), f32, kind="ExternalInput").ap()
    out = nc.dram_tensor("out", (NPAIR_CORE, S, D), f32, kind="ExternalOutput").ap()

    with tile.TileContext(nc) as tc:
        with (
            tc.tile_pool(name="consts", bufs=1) as consts,
            tc.tile_pool(name="qk_in", bufs=3) as qk_in,
            tc.tile_pool(name="kv_in", bufs=3) as kv_in,
            tc.tile_pool(name="pt", bufs=4) as ptp,
            tc.tile_pool(name="cp", bufs=2) as cpp,
            tc.tile_pool(name="msnap", bufs=2) as msnapp,
            tc.tile_pool(name="outsb", bufs=4) as outsb,
            tc.tile_pool(name="ps_m", bufs=1, space="PSUM") as ps_m,
            tc.tile_pool(name="ps_mem", bufs=1, space="PSUM") as ps_mem,
            tc.tile_pool(name="ps_att", bufs=1, space="PSUM") as ps_att,
            tc.tile_pool(name="ps_st", bufs=3, space="PSUM") as ps_st,
        ):
            mkt = consts.tile([128, 128], bf16)
            nc.sync.dma_start(mkt[:], mask[:])
            gst = consts.tile([128, 2 * NPAIR_CORE], f32)
            nc.sync.dma_start(gst[:], gsc[:])
            magic = consts.tile([128, 16], mybir.dt.int32)
            nc.gpsimd.memset(magic[:], 0x7EF311C3)

            for p in range(NPAIR_CORE):
                m_aug = ps_m.tile([D, D + 1], f32)       # [M | norm] accumulator
                m_snap = msnapp.tile([D, D + 1], bf16)
                nc.gpsimd.memset(m_snap[:], 0.0)
                g_col = gst[:, p : p + 1]
                omg_col = gst[:, NPAIR_CORE + p : NPAIR_CORE + p + 1]

                for s in range(NSEG):
                    qkq_t = qk_in.tile([D, 3, SEG], bf16, tag="qkq")
                    nc.sync.dma_start(
                        qkq_t[:],
                        qkq[p, :, :, s * SEG : (s + 1) * SEG].rearrange(
                            "c d n -> d c n"
                        ),
                    )
                    q_t = qkq_t[:, 0, :]
                    k_t = qkq_t[:, 1, :]
                    sq_t = qkq_t[:, 2, :]
                    sk_t = kv_in.tile([128, 8, 64], bf16, tag="sk")
                    nc.sync.dma_start(sk_t[:], skt[p, s].rearrange("p (t d) -> p t d", t=8))
                    v_aug = kv_in.tile([128, 8, 65], bf16, tag="v")
                    nc.sync.dma_start(
                        v_aug[:, :, 0:64], vt[p, s].rearrange("p (t d) -> p t d", t=8)
                    )
                    nc.vector.memset(v_aug[:, :, 64:65], 1.0)

                    # ---- S^T = Kr @ Qr^T (causal chunks), exp, mask diag
                    pt = ptp.tile([128, 8, SEG], bf16)
                    for t in range(8):
                        chunks = []
                        if t < 4:
                            chunks.append((t * 128, 512))
                        chunks.append((max(t * 128, 512), SEG))
                        for (c0, c1) in chunks:
                            st = ps_st.tile([128, 512], f32, tag="st")
                            nc.tensor.matmul(
                                st[:, 0 : c1 - c0],
                                k_t[:, t * 128 : (t + 1) * 128],
                                q_t[:, c0:c1],
                                start=True,
                                stop=True,
                                skip_group_check=True,
                            )
                            nc.scalar.activation(
                                pt[:, t, c0:c1],
                                st[:, 0 : c1 - c0],
                                mybir.ActivationFunctionType.Exp,
                            )
                        nc.vector.tensor_tensor(
                            pt[:, t, t * 128 : (t + 1) * 128],
                            pt[:, t, t * 128 : (t + 1) * 128],
                            mkt[:],
                            op=MULT,
                        )

                    # ---- memory update: M_aug += sigma_k^T @ [v | 1]
                    # (emitted after S^T so it fills PE gaps during exp waits)
                    for t in range(8):
                        nc.tensor.matmul(
                            m_aug[:],
                            sk_t[:, t, :],
                            v_aug[:, t, :],
                            start=(s == 0 and t == 0),
                            stop=(s == NSEG - 1 and t == 7),
                            skip_group_check=True,
                        )

                    # ---- per 128-q chunk: PV and memory retrieval directly in
                    # [q, 65] layout (lhsT = P^T chunk / sigma_q^T chunk)
                    att_big = ps_att.tile([128, 8, 128], f32)
                    mem_big = None
                    if s > 0:
                        mem_big = ps_mem.tile([128, 8, 128], f32)
                    for j in range(8):
                        for t in range(j + 1):
                            nc.tensor.matmul(
                                att_big[:, j, 0 : D + 1],
                                pt[:, t, j * 128 : (j + 1) * 128],
                                v_aug[:, t, :],
                                start=(t == 0),
                                stop=(t == j),
                                skip_group_check=True,
                            )
                        if s > 0:
                            nc.tensor.matmul(
                                mem_big[:, j, 0 : D + 1],
                                sq_t[:, j * 128 : (j + 1) * 128],
                                m_snap[:],
                                start=True, stop=True, skip_group_check=True,
                            )

                    dens = outsb.tile([128, 16], f32, tag="dens")
                    nc.vector.tensor_copy(dens[:, 0:8], att_big[:, :, D])
                    if s > 0:
                        nc.vector.tensor_scalar(
                            dens[:, 8:16], mem_big[:, :, D], EPS, None, op0=ADD
                        )
                    else:
                        nc.vector.memset(dens[:, 8:16], 1.0)
                    # Newton reciprocal: seed from exponent bits, 2 iters
                    recs = outsb.tile([128, 16], f32, tag="recs")
                    nc.vector.tensor_tensor(
                        recs[:].bitcast(mybir.dt.int32), magic[:],
                        dens[:].bitcast(mybir.dt.int32),
                        op=mybir.AluOpType.subtract,
                    )
                    nwt = outsb.tile([128, 16], f32, tag="nwt")
                    for _ in range(2):
                        nc.vector.tensor_tensor(nwt[:], dens[:], recs[:], op=MULT)
                        nc.vector.tensor_scalar(
                            nwt[:], nwt[:], -1.0, 2.0, op0=MULT, op1=ADD
                        )
                        nc.vector.tensor_tensor(recs[:], recs[:], nwt[:], op=MULT)
                    # fold gates into the per-chunk reciprocals
                    recs_a = outsb.tile([128, 8], f32, tag="ra")
                    nc.vector.tensor_scalar(recs_a[:], recs[:, 0:8], omg_col, None, op0=MULT)
                    recs_m = outsb.tile([128, 8], f32, tag="rm")
                    nc.vector.tensor_scalar(recs_m[:], recs[:, 8:16], g_col, None, op0=MULT)

                    o_sb = outsb.tile([128, 8, D], f32, tag="o")
                    if s > 0:
                        t1 = outsb.tile([128, 8, D], f32, tag="t1")
                        nc.vector.tensor_tensor(
                            t1[:], att_big[:, :, 0:D],
                            recs_a[:].unsqueeze(2).broadcast_to([128, 8, D]),
                            op=MULT,
                        )
                        t2 = outsb.tile([128, 8, D], f32, tag="t2")
                        nc.vector.tensor_tensor(
                            t2[:], mem_big[:, :, 0:D],
                            recs_m[:].unsqueeze(2).broadcast_to([128, 8, D]),
                            op=MULT,
                        )
                        nc.vector.tensor_tensor(o_sb[:], t1[:], t2[:], op=ADD)
                    else:
                        # segment 0: memory is empty, output is (1-g)*attn
                        nc.vector.tensor_tensor(
                            o_sb[:], att_big[:, :, 0:D],
                            recs_a[:].unsqueeze(2).broadcast_to([128, 8, D]),
                            op=MULT,
                        )
                    nc.sync.dma_start(
                        out[p, s * SEG : (s + 1) * SEG, :].rearrange(
                            "(jj pp) d -> pp jj d", pp=128
                        ),
                        o_sb[:],
                    )

                    # ---- snapshot memory state for next segment
                    if s < NSEG - 1:
                        m_snap = msnapp.tile([D, D + 1], bf16)
                        nc.vector.tensor_copy(m_snap[:], m_aug[:])

    nc.compile()
    names = dict(qrT="qrT", krT="krT", sqT="sqT", skt="skt", vt="vt",
                 ident="ident", mask="mask", gsc="gsc", out="out")
    _GRAPH_CACHE["nc"] = nc
    _GRAPH_CACHE["names"] = names
    return nc, names


def _host_prep(q, k, v, gate):
    """Produce per-core input maps."""
    cos, sin = _rope_tables()
    P = B * H
    qp = q.reshape(P, NSEG, SEG, D).astype(np.float32)
    kp = k.reshape(P, NSEG, SEG, D).astype(np.float32)
    vp = v.reshape(P, S, D).astype(np.float32)

    qr = _apply_rope_np(qp, cos, sin) * np.float32(1.0 / np.sqrt(D))
    kr = _apply_rope_np(kp, cos, sin)
    sq = np.where(qp > 0, qp + 1.0, np.exp(np.minimum(qp, 0.0))).astype(np.float32)
    sk = np.where(kp > 0, kp + 1.0, np.exp(np.minimum(kp, 0.0))).astype(np.float32)
    # stacked + transposed [P, 3, D, S]
    qkq = np.ascontiguousarray(
        np.stack(
            [qr.reshape(P, S, D), kr.reshape(P, S, D), sq.reshape(P, S, D)],
            axis=1,
        ).transpose(0, 1, 3, 2)
    ).astype(BF16)
    # pre-tiled [pair, seg, 128, 8*64]
    skt = np.ascontiguousarray(
        sk.reshape(P, NSEG, 8, 128, D).transpose(0, 1, 3, 2, 4)
        .reshape(P, NSEG, 128, 512)).astype(BF16)
    vt = np.ascontiguousarray(
        vp.reshape(P, NSEG, 8, 128, D).transpose(0, 1, 3, 2, 4)
        .reshape(P, NSEG, 128, 512)).astype(BF16)

    ident = np.eye(128, dtype=np.float32).astype(BF16)
    mask = np.triu(np.ones((128, 128), dtype=np.float32)).astype(BF16)

    g = 1.0 / (1.0 + np.exp(-gate.reshape(H).astype(np.float64)))
    g = g.astype(np.float32)

    in_maps = []
    for c in range(NCORES):
        pairs = range(c * NPAIR_CORE, (c + 1) * NPAIR_CORE)
        gsc = np.zeros((128, 2 * NPAIR_CORE), dtype=np.float32)
        for i, pr in enumerate(pairs):
            gsc[:, i] = g[pr % H]
            gsc[:, NPAIR_CORE + i] = 1.0 - g[pr % H]
        sl = slice(c * NPAIR_CORE, (c + 1) * NPAIR_CORE)
        in_maps.append({
            "qkq": qkq[sl], "skt": skt[sl], "vt": vt[sl],
            "ident": ident, "mask": mask, "gsc": gsc,
        })
    return in_maps


def kernel(q, k, v, gate, _trace=False):
    from concourse import bass_utils

    nc, _ = _build_graph()
    in_maps = _host_prep(q, k, v, gate)
    res = bass_utils.run_bass_kernel_spmd(
        nc, in_maps, core_ids=list(range(NCORES)), trace=_trace
    )
    outs = [res.results[c]["out"] for c in range(NCORES)]
    full = np.concatenate(outs, axis=0).reshape(B, H, S, D).astype(np.float32)
    if _trace:
        kernel.last_exec_time_ns = res.exec_time_ns
        kernel.last_results = res
    return full
